# revision 80
# baseline (speedup 1.0000x reference)
"""ConvBnA_int kernel for Trainium2 (Bass/Tile), 8 NeuronCores — fp8 DoubleRow.

Problem: y = clip((conv3x3(x, w, pad=1) + t) >> (-n), act_min, act_max).astype(int8)
  x: (32, 128, 56, 56) f32 (integer values 0..127)
  w: (256, 128, 3, 3) f32 (integer values -128..127)

Strategy:
  - Data-parallel over batch: 4 images per core, 8 cores, no communication.
  - Subtractive-Karatsuba limb split in fp8 e4m3 (exact: all limb values are
    small integers, products accumulate exactly in fp32 PSUM):
      x = 16a + b   (a in 0..8,  b in -8..7)
      w = 16c + d   (c in -8..8, d in -8..7)
      x*w = 17*(16*a*c + b*d) - 16*(a-b)*(c-d)     [verified exhaustively]
    Per output tile two PSUM accumulators:
      Q_a = conv(16a, c) + conv(b, d)    -> 9 DoubleRow MMs (same-tap pairs)
      Q_b = conv(a-b, c-d)               -> 5 DoubleRow MMs
      y   = 17*Q_a - 16*Q_b + t, then >>, clamp.
  - fp8 DoubleRow matmul contracts 256 rows (2 slots x 128 cin) per
    instruction at 0.5 cycles/output-col: 14 MMs/tile vs 18 bf16-equivalent.
    Q_a pairs slot0=16a-array with weight c, slot1=b-array with weight d at
    the same tap (slot stride = 4096 = array pitch). Q_b pairs taps
    (k,0)+(k,1) via a pre-shifted copy of (a-b) (slot stride 4096), taps
    (0,2)+(1,2) via slot stride 64 (row pitch), and tap (2,2) with a zeroed
    slot1.
  - Images live in SBUF as [128, 2, 64x64] fp8 canvases (zero border, row
    pitch 64 so DMA runs are >=512B and slot strides stay 16-aligned; the
    DoubleRow moving-operand slot stride must be a multiple of 16 — probed:
    stride 1 gives wrong results on hardware).
  - Post per tile [128, 8x56]: ACT u=f32(17*Q_a+t), DVE scalar_tensor_tensor
    z=i32((Q_b*-16)+u), DVE shift, GPSIMD clamp->int8 (engines balanced so
    the PE stream of DoubleRow MMs is the critical path).
  - Startup: weights are couttile-major in DRAM so c=0's half loads first;
    the serialized DMA device order is [wqa-c0, xa-rows0..17, wqb-c0,
    xb-rows0..18, cst, ...] and the first two units' Qa MMs are hoisted
    ahead of their Qb so the PE's in-order queue never waits on the
    later-arriving Qb inputs.  First real MM ~4.5us, gapless after.
  - Tail: the last three units land in dedicated always-alive tiles; st6 is
    split into two 4-row sub-units (last clamp on DVE) so the closing chain
    covers 224 cols, with one store for (st4,st5) and one for st6.
  - Exactness: fp8 operands/products are exact; fp32 rounding can only occur
    for |pre-shift| > 2^24, which after >>5..10 lands far beyond the clamp
    bounds, so the int8 output is still exact (verified 0 mismatches).
"""

import numpy as np
import ml_dtypes

B, CIN, COUT, H, W, K = 32, 128, 256, 56, 56, 3
N_CORES = 8
B_LOC = B // N_CORES          # 4 images per core
PW = 64                       # padded row pitch (56 + 2 border + 6 pad)
PADN = 64 * PW                # 4096 elems per slot per partition
ROWS_PER_TILE = 8
NTILE = H // ROWS_PER_TILE    # 7 spatial tiles
TILE_N = ROWS_PER_TILE * W    # 448
NQ = H * W                    # 3136
CTILES = COUT // 128          # 2

F8 = ml_dtypes.float8_e4m3
WARM_MMS = 18       # warm-up matmuls ahead of the real stream
WARM_N = 448        # output cols per warm-up matmul
TAIL_DVE_K = 1      # clamp of the last K units runs on DVE instead of Pool
TAIL_SPLIT = False  # split the final unit's post into two half-tiles
TAIL_V2 = True      # tail units as two 4-row sub-units each
TAIL_UNITS = 1      # how many trailing units get the sub-unit treatment
TAIL_KV = True      # st4/st5/st6 land in dedicated tiles with direct stores
TAIL_SPLIT6 = False  # store st6's two sub-units separately
# 4-product single-PSUM tail (no STT combine): reached 82592ns in sim but the
# ACT f32->i32 path introduced +-2 LSB errors on ~0.03% of elements (rel
# 1.56e-2 — passes a global-max gate but not an elementwise one), so it
# stays off in favor of the exact Karatsuba tail.
TAIL_P4 = False
SGRP = 2            # spatial tiles per output store
LAST_STORE_ENG = "sync"  # queue for the final unit's store
ZZ_BUFS = 4
HOIST = 2           # leading units whose Qa issues ahead of their Qb
# b=0 x-canvas chunk bounds (elems per slot); rows r -> (r+1)*PW covers
# canvas rows <= r.  xa taps read rows <=57, xb's shifted slot rows <=58.
XA_BNDS = [0, 18 * 64, 42 * 64, 58 * 64]
XB_BNDS = [0, 19 * 64, 43 * 64, 59 * 64]
PS_BUFS = 4         # psA and psB are separate tags: 4 banks each

_CACHE = {}


def _build_nc():
    import concourse.mybir as mybir
    import concourse.tile as tile
    from concourse import bacc
    from concourse.ap import AP

    dt = mybir.dt
    DR = mybir.MatmulPerfMode.DoubleRow
    nc = bacc.Bacc(
        "TRN2", target_bir_lowering=False, debug=False, num_devices=N_CORES
    )

    xa = nc.dram_tensor("xa", [B_LOC, CIN, 2 * PADN], dt.float8e4, kind="ExternalInput")
    xb = nc.dram_tensor("xb", [B_LOC, CIN, 2 * PADN], dt.float8e4, kind="ExternalInput")
    wqa = nc.dram_tensor("wqa", [CIN, CTILES * 9 * 2 * 128], dt.float8e4, kind="ExternalInput")
    wqb = nc.dram_tensor("wqb", [CIN, CTILES * 5 * 2 * 128], dt.float8e4, kind="ExternalInput")
    wq6 = nc.dram_tensor("wq6", [CIN, 9 * 4 * 128], dt.float8e4, kind="ExternalInput")
    cst = nc.dram_tensor("cst", [128, 4 * CTILES + 4], dt.int32, kind="ExternalInput")
    out = nc.dram_tensor("out", [B_LOC, COUT, NQ], dt.int8, kind="ExternalOutput")

    with tile.TileContext(nc) as tc:
        with (
            tc.tile_pool(name="const", bufs=1) as cpool,
            tc.tile_pool(name="xin", bufs=2) as xin_pool,
            tc.tile_pool(name="psum", bufs=PS_BUFS, space="PSUM") as pspool,
            tc.tile_pool(name="uv", bufs=8) as uv_pool,
            tc.tile_pool(name="zz", bufs=ZZ_BUFS) as zz_pool,
            tc.tile_pool(name="o8", bufs=6) as o8_pool,
        ):
            # weights are couttile-major in DRAM so the c=0 half loads in one
            # contiguous DMA.  Startup order on the (serialized) DMA device:
            #   wqa-c0, xa-chunk0, wqb-c0, xb-chunk0, cst, xa1, xb1, xa2,
            #   xb2, wqa-c1, wqb-c1, b1.. image loads
            # All startup DMAs ride the sync (SP) queue: it has the cheapest
            # SEQ/HWDGE path, and transfers serialize on one DMA device
            # anyway, so one queue keeps the order deterministic.
            wqa_sb = cpool.tile([CIN, CTILES, 9, 2, 128], dt.float8e4)
            wqa_v = wqa[:, :].rearrange("p (c t s m) -> p c t s m", c=CTILES, t=9, s=2)
            wqb_sb = cpool.tile([CIN, CTILES, 5, 2, 128], dt.float8e4)
            wqb_v = wqb[:, :].rearrange("p (c t s m) -> p c t s m", c=CTILES, t=5, s=2)
            nc.sync.dma_start(wqa_sb[:, 0], wqa_v[:, 0])
            cst_sb = cpool.tile([128, 4 * CTILES + 4], dt.int32)
            tv_sb = cst_sb[:, 0 * CTILES : 1 * CTILES].bitcast(mybir.dt.float32)
            sv_sb = cst_sb[:, 1 * CTILES : 2 * CTILES]
            amin_sb = cst_sb[:, 2 * CTILES : 3 * CTILES].bitcast(mybir.dt.float32)
            amax_sb = cst_sb[:, 3 * CTILES : 4 * CTILES].bitcast(mybir.dt.float32)
            # always-alive landing tiles for the last three units' outputs
            # (one 2-tile group for st4/st5, one for the split st6)
            tail_o8 = cpool.tile([128, ROWS_PER_TILE, W], dt.int8)
            t45_o8 = cpool.tile([128, 2 * ROWS_PER_TILE, W], dt.int8)
            # final unit's 4-product weights: per tap [16c | d | d | 16c] for
            # couttile 1, so one PSUM accumulates the whole conv:
            #   (16a vs 16c)+(b vs d) = 256ac+bd ; (16a vs d)+(b vs 16c) =
            #   16ad+16bc — all values e4m3-exact
            wq6_sb = cpool.tile([CIN, 9, 4, 128], dt.float8e4)


            # PE clock warm-up: a trickle of tiny dependency-free MMs keeps
            # the PE continuously busy from t~0 so the p-state ramp (3us)
            # completes before the real MM stream starts
            junk = cpool.tile([CIN, 2, ROWS_PER_TILE * W], dt.float8e4)
            nc.gpsimd.memset(junk[:].bitcast(mybir.dt.int32), 0.0)

            for b in range(B_LOC):
                xa_sb = xin_pool.tile([CIN, 2, PADN], dt.float8e4)
                xb_sb = xin_pool.tile([CIN, 2, PADN], dt.float8e4)

                def chunk(t_sb, t_dr, lo, hi, _b=b):
                    # both slots in one strided DMA: fewer DGEs and no
                    # sub-500ns transfer floors
                    nc.sync.dma_start(
                        t_sb[:, :, lo:hi],
                        t_dr[_b, :, :].rearrange("p (s n) -> p s n", s=2)
                        [:, :, lo:hi],
                    )

                if b == 0:
                    # startup order on the serialized DMA device: wqa-c0 was
                    # issued first (above); then the smallest x chunks that
                    # cover the first two spatial tiles, wqb-c0, cst, the
                    # remaining b0 chunks, and the c=1 weights.
                    chunk(xa_sb, xa, XA_BNDS[0], XA_BNDS[1])
                    nc.sync.dma_start(wqb_sb[:, 0], wqb_v[:, 0])
                    chunk(xb_sb, xb, XB_BNDS[0], XB_BNDS[1])
                    nc.sync.dma_start(cst_sb[:], cst[:, :])
                    for ci in range(1, 3):
                        chunk(xa_sb, xa, XA_BNDS[ci], XA_BNDS[ci + 1])
                        chunk(xb_sb, xb, XB_BNDS[ci], XB_BNDS[ci + 1])
                    nc.sync.dma_start(wqa_sb[:, 1], wqa_v[:, 1])
                    nc.sync.dma_start(wqb_sb[:, 1], wqb_v[:, 1])
                    if TAIL_P4:
                        nc.sync.dma_start(
                            wq6_sb[:],
                            wq6[:, :].rearrange("p (t s m) -> p t s m", t=9, s=4),
                        )
                else:
                    chunk(xa_sb, xa, 0, XA_BNDS[-1])
                    chunk(xb_sb, xb, 0, XB_BNDS[-1])

                xav = xa_sb.rearrange("p s (h w) -> p s h w", w=PW)
                xbflat = xb_sb[:]
                pstride = xbflat.ap[0][0]

                def mkap(off, sstride, nr, _t=xbflat):
                    return AP(
                        _t.tensor, off,
                        [[pstride, CIN], [sstride, 2],
                         [PW, nr], [1, W]],
                    )

                def do_qa(c, st, r0=0, r1=ROWS_PER_TILE, warm=False):
                    h0 = st * ROWS_PER_TILE + r0
                    nr = r1 - r0
                    psA = pspool.tile([128, nr, W], dt.float32)
                    if warm:
                        wn = min(WARM_N, nr * W)
                        for _ in range(WARM_MMS):
                            nc.tensor.matmul(
                                psA[:].rearrange("p h w -> p (h w)")[:, 0:wn],
                                junk[:, :, 0:128],
                                junk[:, :, 0:wn],
                                start=True, stop=True,
                                perf_mode=DR, skip_group_check=True,
                            )
                    for t9 in range(9):
                        kh, kw = divmod(t9, K)
                        nc.tensor.matmul(
                            psA[:],
                            wqa_sb[:, c, t9],
                            xav[:, :, h0 + kh : h0 + kh + nr,
                                kw : kw + W],
                            start=(t9 == 0), stop=(t9 == 8),
                            perf_mode=DR,
                        )
                    return psA

                def do_qb(c, st, r0=0, r1=ROWS_PER_TILE):
                    h0 = st * ROWS_PER_TILE + r0
                    nr = r1 - r0
                    psB = pspool.tile([128, nr, W], dt.float32)
                    qb_movs = [
                        (h0 * PW, PADN),            # taps (0,0)+(0,1)
                        ((h0 + 1) * PW, PADN),      # taps (1,0)+(1,1)
                        ((h0 + 2) * PW, PADN),      # taps (2,0)+(2,1)
                        (h0 * PW + 2, PW),          # taps (0,2)+(1,2)
                        ((h0 + 2) * PW + 2, PW),    # tap (2,2) + zero slot
                    ]
                    for t5, (off, sstride) in enumerate(qb_movs):
                        nc.tensor.matmul(
                            psB[:],
                            wqb_sb[:, c, t5],
                            mkap(off, sstride, nr),
                            start=(t5 == 0), stop=(t5 == 4),
                            perf_mode=DR,
                        )
                    return psB

                o8_hold = [None]

                def do_post(c, st, psA, psB, _b=b, force_store=False):
                    # u = f32(17*Q_a + t)       [ACT]
                    # z = i32((Q_b*-16) + u)    [DVE STT, reads PSUM]
                    # sh = z >> sv              [DVE]
                    # o8 = clamp(sh) -> int8    [GPSIMD; DVE for tail]
                    ui = (_b * CTILES + c) * NTILE + st
                    units_left = B_LOC * CTILES * NTILE - 1 - ui
                    is_last = units_left == 0
                    # batch stores in groups of SGRP spatial tiles
                    if st % SGRP == 0:
                        o8_hold[0] = o8_pool.tile(
                            [128, SGRP * ROWS_PER_TILE, W], dt.int8, name="o8"
                        )
                    o8 = o8_hold[0]
                    half = st % SGRP
                    rr = (
                        [(0, 4), (4, 8)]
                        if (TAIL_SPLIT and is_last)
                        else [(0, ROWS_PER_TILE)]
                    )
                    for r0, r1 in rr:
                        nr = r1 - r0
                        u32 = uv_pool.tile([128, nr, W], dt.float32, name="u32")
                        nc.scalar.activation(
                            u32[:], psA[:, r0:r1],
                            mybir.ActivationFunctionType.Identity,
                            bias=tv_sb[:, c : c + 1], scale=17.0,
                        )
                        z32 = zz_pool.tile([128, nr, W], dt.int32, name="z32")
                        nc.vector.scalar_tensor_tensor(
                            z32[:], psB[:, r0:r1], -16.0, u32[:],
                            mybir.AluOpType.mult, mybir.AluOpType.add,
                        )
                        sh32 = zz_pool.tile([128, nr, W], dt.int32, name="sh32")
                        nc.vector.tensor_scalar(
                            sh32[:], z32[:],
                            sv_sb[:, c : c + 1], None,
                            mybir.AluOpType.arith_shift_right,
                        )
                        clamp_eng = (
                            nc.vector if units_left < TAIL_DVE_K else nc.gpsimd
                        )
                        clamp_eng.tensor_scalar(
                            o8[:, half * ROWS_PER_TILE + r0
                               : half * ROWS_PER_TILE + r1],
                            sh32[:],
                            amax_sb[:, c : c + 1], amin_sb[:, c : c + 1],
                            mybir.AluOpType.min, mybir.AluOpType.max,
                        )
                        if TAIL_SPLIT and is_last:
                            lo = st * TILE_N + r0 * W
                            nc.sync.dma_start(
                                out[_b, c * 128 : (c + 1) * 128,
                                    lo : lo + nr * W]
                                .rearrange("p (h w) -> p h w", w=W),
                                o8[:, half * ROWS_PER_TILE + r0
                                   : half * ROWS_PER_TILE + r1],
                            )
                    if (
                        st % SGRP == SGRP - 1 or st == NTILE - 1 or force_store
                    ) and not (TAIL_SPLIT and is_last):
                        npair = st % SGRP + 1
                        lo = (st - npair + 1) * TILE_N
                        seng = (
                            getattr(nc, LAST_STORE_ENG)
                            if units_left == 0 else nc.sync
                        )
                        seng.dma_start(
                            out[_b, c * 128 : (c + 1) * 128,
                                lo : lo + npair * TILE_N]
                            .rearrange("p (h w) -> p h w", w=W),
                            o8[:, : npair * ROWS_PER_TILE],
                        )

                unit_seq = [(c, st) for c in range(CTILES) for st in range(NTILE)]
                if b == 0 and HOIST > 0:
                    # hoist the first HOIST units' Qa ahead of their Qb so the
                    # PE's in-order stream isn't blocked on the later-arriving
                    # Qb inputs (xb chunk0 / wqb-c0 land after xa/wqa-c0)
                    held = []
                    for i, (c, st) in enumerate(unit_seq[:HOIST]):
                        held.append(do_qa(c, st, warm=(i == 0)))
                    for i, (c, st) in enumerate(unit_seq[:HOIST]):
                        psB = do_qb(c, st)
                        do_post(c, st, held[i], psB)
                    rest = unit_seq[HOIST:]
                else:
                    rest = unit_seq
                for (c, st) in rest:
                    tail_c = b == B_LOC - 1 and c == CTILES - 1
                    if TAIL_KV and tail_c and st in (NTILE - 3, NTILE - 2):
                        # (st4, st5): normal 8-row units landing in t45_o8;
                        # st5's shift+clamp move to Pool so DVE is clear for
                        # the final unit's chain; their store fires via the
                        # queue-0 trigger right after st5's clamp
                        psA = do_qa(c, st)
                        psB = do_qb(c, st)
                        half = st - (NTILE - 3)
                        u32 = uv_pool.tile(
                            [128, ROWS_PER_TILE, W], dt.float32, name="u32"
                        )
                        nc.scalar.activation(
                            u32[:], psA[:],
                            mybir.ActivationFunctionType.Identity,
                            bias=tv_sb[:, c : c + 1], scale=17.0,
                        )
                        z32 = zz_pool.tile(
                            [128, ROWS_PER_TILE, W], dt.int32, name="z32"
                        )
                        nc.vector.scalar_tensor_tensor(
                            z32[:], psB[:], -16.0, u32[:],
                            mybir.AluOpType.mult, mybir.AluOpType.add,
                        )
                        sh32 = zz_pool.tile(
                            [128, ROWS_PER_TILE, W], dt.int32, name="sh32"
                        )
                        nc.vector.tensor_scalar(
                            sh32[:], z32[:],
                            sv_sb[:, c : c + 1], None,
                            mybir.AluOpType.arith_shift_right,
                        )
                        # with the P4 tail DVE has no STT work left, so
                        # st5's clamp runs there (294 vs 717 on Pool) and the
                        # t45 store's HWDGE slot clears before st6 needs it
                        cl45 = (
                            nc.vector if (TAIL_P4 and st == NTILE - 2)
                            else nc.gpsimd
                        )
                        cl45.tensor_scalar(
                            t45_o8[:, half * ROWS_PER_TILE
                                   : (half + 1) * ROWS_PER_TILE],
                            sh32[:],
                            amax_sb[:, c : c + 1], amin_sb[:, c : c + 1],
                            mybir.AluOpType.min, mybir.AluOpType.max,
                        )
                        if st == NTILE - 2:
                            lo45 = (NTILE - 3) * TILE_N
                            nc.sync.dma_start(
                                out[b, c * 128 : (c + 1) * 128,
                                    lo45 : lo45 + 2 * TILE_N]
                                .rearrange("p (h w) -> p h w", w=W),
                                t45_o8[:],
                            )
                        continue
                    is_tail = (
                        TAIL_V2 and b == B_LOC - 1 and c == CTILES - 1
                        and st >= NTILE - TAIL_UNITS
                    )
                    if not is_tail:
                        psA = do_qa(c, st, warm=False)
                        psB = do_qb(c, st)
                        do_post(c, st, psA, psB)
                        continue
                    # tail units: two half-height sub-units each, so the
                    # closing DVE chain drains in 224-col pieces against the
                    # MM stream instead of serializing 448-col ops at the end
                    last_unit = st == NTILE - 1
                    if TAIL_P4:
                        # two half-height sub-units, each a single-PSUM
                        # 18-MM 4-product pass (2 MMs per tap)
                        def do_q4(r0, r1):
                            h0 = st * ROWS_PER_TILE + r0
                            nr = r1 - r0
                            # reuse the psA tag so no extra PSUM banks
                            psA = pspool.tile([128, nr, W], dt.float32)
                            for t9 in range(9):
                                kh, kw = divmod(t9, K)
                                mov = xav[:, :, h0 + kh : h0 + kh + nr,
                                          kw : kw + W]
                                for j in range(2):
                                    nc.tensor.matmul(
                                        psA[:],
                                        wq6_sb[:, t9, 2 * j : 2 * j + 2],
                                        mov,
                                        start=(t9 == 0 and j == 0),
                                        stop=(t9 == 8 and j == 1),
                                        perf_mode=DR,
                                    )
                            return psA
                        pQ1 = do_q4(0, 6)
                        pQ2 = do_q4(6, 8)
                        o8 = tail_o8
                        for (pQ, r0, r1) in ((pQ1, 0, 6), (pQ2, 6, 8)):
                            nr = r1 - r0
                            # u = i32(Q + t)  [ACT writes int32 directly]
                            ui = uv_pool.tile([128, nr, W], dt.int32,
                                              name="ui")
                            nc.scalar.activation(
                                ui[:], pQ[:],
                                mybir.ActivationFunctionType.Identity,
                                bias=tv_sb[:, c : c + 1], scale=1.0,
                            )
                            sh32 = zz_pool.tile([128, nr, W], dt.int32,
                                                name="sh32")
                            nc.vector.tensor_scalar(
                                sh32[:], ui[:],
                                sv_sb[:, c : c + 1], None,
                                mybir.AluOpType.arith_shift_right,
                            )
                            nc.vector.tensor_scalar(
                                o8[:, r0:r1], sh32[:],
                                amax_sb[:, c : c + 1], amin_sb[:, c : c + 1],
                                mybir.AluOpType.min, mybir.AluOpType.max,
                            )
                        lo = st * TILE_N
                        nc.sync.dma_start(
                            out[b, c * 128 : (c + 1) * 128, lo : lo + TILE_N]
                            .rearrange("p (h w) -> p h w", w=W),
                            o8[:, 0:ROWS_PER_TILE],
                        )
                        continue
                    pA1 = do_qa(c, st, 0, 4)
                    pB1 = do_qb(c, st, 0, 4)
                    pA2 = do_qa(c, st, 4, 8)
                    pB2 = do_qb(c, st, 4, 8)
                    if TAIL_KV and last_unit:
                        o8 = tail_o8
                    else:
                        o8 = o8_pool.tile(
                            [128, SGRP * ROWS_PER_TILE, W], dt.int8, name="o8"
                        )
                    for (pA, pB, r0, r1) in (
                        (pA1, pB1, 0, 4),
                        (pA2, pB2, 4, 8),
                    ):
                        nr = r1 - r0
                        last_sub = last_unit and r1 == 8
                        u32 = uv_pool.tile([128, nr, W], dt.float32, name="u32")
                        nc.scalar.activation(
                            u32[:], pA[:],
                            mybir.ActivationFunctionType.Identity,
                            bias=tv_sb[:, c : c + 1], scale=17.0,
                        )
                        z32 = zz_pool.tile([128, nr, W], dt.int32, name="z32")
                        nc.vector.scalar_tensor_tensor(
                            z32[:], pB[:], -16.0, u32[:],
                            mybir.AluOpType.mult, mybir.AluOpType.add,
                        )
                        sh32 = zz_pool.tile([128, nr, W], dt.int32, name="sh32")
                        nc.vector.tensor_scalar(
                            sh32[:], z32[:],
                            sv_sb[:, c : c + 1], None,
                            mybir.AluOpType.arith_shift_right,
                        )
                        ceng = nc.vector if last_sub else nc.gpsimd
                        ceng.tensor_scalar(
                            o8[:, r0:r1], sh32[:],
                            amax_sb[:, c : c + 1], amin_sb[:, c : c + 1],
                            mybir.AluOpType.min, mybir.AluOpType.max,
                        )
                        if TAIL_KV and last_unit and TAIL_SPLIT6:
                            # store each half as soon as its clamp lands so
                            # the final DMA covers only 224 cols
                            lo6 = st * TILE_N + r0 * W
                            nc.sync.dma_start(
                                out[b, c * 128 : (c + 1) * 128,
                                    lo6 : lo6 + nr * W]
                                .rearrange("p (h w) -> p h w", w=W),
                                o8[:, r0:r1],
                            )
                    lo = st * TILE_N
                    if TAIL_KV and last_unit:
                        if not TAIL_SPLIT6:
                            nc.sync.dma_start(
                                out[b, c * 128 : (c + 1) * 128,
                                    lo : lo + TILE_N]
                                .rearrange("p (h w) -> p h w", w=W),
                                o8[:, 0:ROWS_PER_TILE],
                            )
                    else:
                        nc.sync.dma_start(
                            out[b, c * 128 : (c + 1) * 128, lo : lo + TILE_N]
                            .rearrange("p (h w) -> p h w", w=W),
                            o8[:, 0:ROWS_PER_TILE],
                        )
    nc.compile()
    return nc


def _f8_lut():
    # uint8 bit patterns for exact small-int -> fp8 e4m3 conversion
    vals = np.arange(-16, 241, dtype=np.int32)
    lut = np.zeros(257, dtype=np.uint8)
    lut[:] = vals.astype(np.float32).astype(F8).view(np.uint8)
    return lut


def _prep_inputs(x, weight, t, n, act_min, act_max):
    lut = _f8_lut()

    def to_f8(ints):  # int array (>= -16) -> fp8 bytes
        return lut[ints + 16].view(F8)

    xi = x.astype(np.int32)
    a = (xi + 8) >> 4                 # 0..8
    bb = xi - (a << 4)                # -8..7

    def canvas(vals):  # [B, CIN, 56, 56] int32 -> [B, CIN, PADN] int32
        cv = np.zeros((B, CIN, 64, PW), dtype=np.int32)
        cv[:, :, 1 : H + 1, 1 : W + 1] = vals
        return cv.reshape(B, CIN, PADN)

    A2 = canvas(a << 4)
    Bc = canvas(bb)
    D2 = canvas(a - bb)               # -7..16
    D2s = np.zeros_like(D2)
    D2s[:, :, : PADN - 1] = D2[:, :, 1:]
    xa = to_f8(np.concatenate([A2, Bc], axis=2))
    xb = to_f8(np.concatenate([D2, D2s], axis=2))

    wi = weight.astype(np.int32)      # [COUT, CIN, 3, 3]
    c = (wi + 8) >> 4                 # -8..8
    d = wi - (c << 4)                 # -8..7
    e = c - d                         # -15..16

    # wqa[p, tap, slot, ct, m]: slot0 = c (vs 16a), slot1 = d (vs b)
    wqa = np.zeros((CIN, 9, 2, CTILES, 128), dtype=np.int32)
    wqb = np.zeros((CIN, 5, 2, CTILES, 128), dtype=np.int32)
    cT = c.reshape(CTILES, 128, CIN, K, K).transpose(2, 3, 4, 0, 1)   # [p,kh,kw,ct,m]
    dT = d.reshape(CTILES, 128, CIN, K, K).transpose(2, 3, 4, 0, 1)
    eT = e.reshape(CTILES, 128, CIN, K, K).transpose(2, 3, 4, 0, 1)
    for t9 in range(9):
        kh, kw = divmod(t9, K)
        wqa[:, t9, 0] = cT[:, kh, kw]
        wqa[:, t9, 1] = dT[:, kh, kw]
    for k in range(3):
        wqb[:, k, 0] = eT[:, k, 0]
        wqb[:, k, 1] = eT[:, k, 1]
    wqb[:, 3, 0] = eT[:, 0, 2]
    wqb[:, 3, 1] = eT[:, 1, 2]
    wqb[:, 4, 0] = eT[:, 2, 2]
    # wqb[:, 4, 1] stays zero
    # couttile-major DRAM layout [p, c, t, s, m] so each couttile's weights
    # load in one contiguous DMA
    wqa_f8 = to_f8(np.ascontiguousarray(wqa.transpose(0, 3, 1, 2, 4)).reshape(CIN, -1))
    wqb_f8 = to_f8(np.ascontiguousarray(wqb.transpose(0, 3, 1, 2, 4)).reshape(CIN, -1))

    # final-unit 4-product weights (couttile 1): per tap [16c | d | d | 16c];
    # 16c reaches +-128 (outside the LUT) but is e4m3-exact, so convert
    # directly
    wq6v = np.zeros((CIN, 9, 4, 128), dtype=np.int32)
    for t9 in range(9):
        kh, kw = divmod(t9, K)
        wq6v[:, t9, 0] = 16 * cT[:, kh, kw, 1]
        wq6v[:, t9, 1] = dT[:, kh, kw, 1]
        wq6v[:, t9, 2] = dT[:, kh, kw, 1]
        wq6v[:, t9, 3] = 16 * cT[:, kh, kw, 1]
    wq6_f8 = wq6v.reshape(CIN, -1).astype(np.float32).astype(F8)

    def percore_vec(v, dtype):
        return np.ascontiguousarray(v.reshape(CTILES, 128).T).astype(dtype)

    cstv = np.zeros((128, 4, CTILES), dtype=np.int32)
    cstv[:, 0] = percore_vec(t, np.float32).view(np.int32)
    cstv[:, 1] = percore_vec(-n, np.int32)
    cstv[:, 2] = percore_vec(act_min, np.float32).view(np.int32)
    cstv[:, 3] = percore_vec(act_max, np.float32).view(np.int32)
    # ctx indices for the tail kv_writeback stores: offsets are folded into
    # the descriptor AP bases, so all batch entries write at ctx 0
    kidx = np.zeros((128, 4), dtype=np.int32)
    return xa, xb, wqa_f8, wqb_f8, wq6_f8, np.concatenate(
        [cstv.reshape(128, -1), kidx], axis=1
    )


def kernel(x, weight, t, n, act_min, act_max):
    from concourse.bass_utils import run_bass_kernel_spmd

    xa, xb, wqa, wqb, wq6, cstv = _prep_inputs(x, weight, t, n, act_min, act_max)

    if "nc" not in _CACHE:
        _CACHE["nc"] = _build_nc()
    nc = _CACHE["nc"]

    in_maps = []
    for c in range(N_CORES):
        in_maps.append(
            dict(
                xa=xa[c * B_LOC : (c + 1) * B_LOC],
                xb=xb[c * B_LOC : (c + 1) * B_LOC],
                wqa=wqa, wqb=wqb, wq6=wq6, cst=cstv,
            )
        )
    res = run_bass_kernel_spmd(nc, in_maps, core_ids=list(range(N_CORES)))
    outs = [r["out"] for r in res.results]
    full = np.concatenate(outs, axis=0)              # [32, 256, 3136]
    return np.ascontiguousarray(full.reshape(B, COUT, H, W))

